# revision 27
# baseline (speedup 1.0000x reference)
"""Trainium2 Bass kernel for GNN message-passing attention block.

Sharding: 8 cores = 2 batches x 4 query-blocks of 256 queries. Each core
computes GroupNorm + K/V projections for its batch over all 1024 nodes
(needed since any query may attend anywhere), and Q/attention/output for
its 256-query block.

On-device everything is dense in bf16 (fp32 PSUM accumulation):
  GroupNorm -> Q/K/V projections (PE) -> dense scores via zero-padded-Q
  matmuls with full-128 contraction (PE) -> exp (ACT) -> multiply by
  host-precomputed multiplicity/validity mask M[j,q] (DVE) -> V^T @ A
  matmul with an appended ones-column giving both the weighted sum and
  the softmax normalizer Z (PE) -> divide -> output projection + residual.

The sparse gather of the reference collapses into the dense mask M since
duplicate neighbor indices share a score: combined softmax weight is
multiplicity * exp(s) / Z.  K-bias is skipped (softmax shift-invariant);
V-bias is folded into the output bias.

All matmuls keep PE tile_position (0,0): alternating 64-row tile configs
(stationary operands at partition offset 64) execute fine in CoreSim but
fail on hardware.  Scores instead contract over the full 128-channel
chunk against a Q buffer with zeroed off-head quadrants.
"""

import sys

if "/opt/trn_rl_repo" not in sys.path:
    sys.path.insert(0, "/opt/trn_rl_repo")

import numpy as np
import ml_dtypes

import concourse.bacc as bacc
import concourse.mybir as mybir
import concourse.tile as tile
from contextlib import ExitStack

F32 = mybir.dt.float32
BF16 = mybir.dt.bfloat16
AF = mybir.ActivationFunctionType
ALU = mybir.AluOpType
AX = mybir.AxisListType
NPBF = ml_dtypes.bfloat16

B, C, N, K, H, DH, NG = 2, 512, 1024, 64, 8, 64, 32
NQ = 256            # queries per core (4 blocks x 2 batches = 8 cores)
NCHUNK = C // 128   # 4 channel chunks
NJC = N // 128      # 8 key-node chunks
EPS = 1e-6
GSIZE = (C // NG) * N  # elements per group (one batch) = 16*1024

_CACHE = {}


def _emit(tc, nc, t):
    """Emit the per-core program. t: dict of DRAM APs."""
    ctx = t["ctx"]
    P = 128

    wpool = ctx.enter_context(tc.tile_pool(name="weights", bufs=1))
    cpool = ctx.enter_context(tc.tile_pool(name="consts", bufs=1))
    xpool = ctx.enter_context(tc.tile_pool(name="x", bufs=1))
    hpool = ctx.enter_context(tc.tile_pool(name="h", bufs=1))
    kvpool = ctx.enter_context(tc.tile_pool(name="kv", bufs=1))
    apool = ctx.enter_context(tc.tile_pool(name="attn", bufs=1))
    spool = ctx.enter_context(tc.tile_pool(name="scratch", bufs=2))
    smallp = ctx.enter_context(tc.tile_pool(name="small", bufs=2))
    opool = ctx.enter_context(tc.tile_pool(name="out", bufs=2))
    # Single 2-bank-tile double-buffered pool for all matmul outputs except
    # po: 4 banks here + 4 banks for po = all 8 PSUM banks.
    pp = ctx.enter_context(tc.tile_pool(name="psum", bufs=2, space="PSUM"))
    ppo = ctx.enter_context(tc.tile_pool(name="psum_o", bufs=1, space="PSUM"))

    # ---- input DMAs: merged transfers spread over 4 DGE queues; small
    # constants first so GroupNorm stats never wait behind the weights.
    consts = cpool.tile([P, 16], F32)  # bq|gamma|beta|bo_eff (4 cols each)
    gfwd = cpool.tile([P, 8], F32)
    gbwd = cpool.tile([8, P], F32)
    # All input DMAs issue from SP in priority order: an engine's compute
    # stalls behind its own pending DMA issues (HWDGE descriptor processing
    # is ~630ns each, serialized), and SP has no compute to stall.
    xall = xpool.tile([P, NCHUNK, N], BF16, tag="x", name="xall")
    xdram = t["x16"].rearrange("(m p) n -> p m n", p=P)
    xqall = xpool.tile([P, NCHUNK, NQ], F32, tag="xq", name="xqall")
    wqa = wpool.tile([P, NCHUNK, C], BF16, tag="wq", name="wqa")
    wka = wpool.tile([P, NCHUNK, C], BF16, tag="wk", name="wka")
    wva = wpool.tile([P, NCHUNK, C], BF16, tag="wv", name="wva")
    woa = wpool.tile([P, NCHUNK, C], BF16, tag="wo", name="woa")
    mall = cpool.tile([P, NJC, NQ], BF16, tag="mall", name="mall")
    for m in range(NCHUNK):
        nc.sync.dma_start(xall[:, m:m + 1, :], xdram[:, m:m + 1, :])
    nc.sync.dma_start(consts[:], t["consts"][:, :])
    nc.sync.dma_start(gfwd[:], t["gfwd"][:, :])
    nc.sync.dma_start(gbwd[:], t["gbwd"][:, :])
    nc.sync.dma_start(xqall[:], t["xq"].rearrange("(m p) q -> p m q", p=P))
    nc.sync.dma_start(wqa[:], t["wqT"].rearrange("(k p) o -> p k o", p=P))
    nc.sync.dma_start(wka[:], t["wkT"].rearrange("(k p) o -> p k o", p=P))
    nc.sync.dma_start(mall[:], t["mmask"].rearrange("j p q -> p j q"))
    nc.sync.dma_start(wva[:], t["wvT"].rearrange("(k p) o -> p k o", p=P))
    nc.sync.dma_start(woa[:], t["woT"].rearrange("(k p) o -> p k o", p=P))
    xsb = [xall[:, m, :] for m in range(NCHUNK)]
    xq = [xqall[:, m, :] for m in range(NCHUNK)]
    msb = [mall[:, jc, :] for jc in range(NJC)]
    wq = [wqa[:, i, :] for i in range(NCHUNK)]
    wk = [wka[:, i, :] for i in range(NCHUNK)]
    wv = [wva[:, i, :] for i in range(NCHUNK)]
    wo = [woa[:, i, :] for i in range(NCHUNK)]
    ones1 = cpool.tile([1, DH], BF16)
    nc.vector.memset(ones1[:], 1.0)

    # ---- GroupNorm statistics (bf16 x, fp32 accumulation) ----
    ssq = smallp.tile([P, 8], F32, tag="ssq", name="ssq")  # cols 0-3 sum, 4-7 sum sq
    for m in range(NCHUNK):
        junk = spool.tile([P, N], BF16, tag="junk", name="junk")
        nc.scalar.activation(junk[:], xsb[m], AF.Square,
                             accum_out=ssq[:, 4 + m:5 + m])
        nc.vector.tensor_reduce(ssq[:, m:m + 1], xsb[m], AX.X, ALU.add)
    gs = pp.tile([8, 8], F32, tag="mm", name="gs")
    nc.tensor.matmul(gs[:], gfwd[:], ssq[:], start=True, stop=True)
    mu = smallp.tile([8, 8], F32, tag="mu", name="mu")  # cols 0-3 mean, 4-7 E[x^2]
    nc.scalar.activation(mu[:], gs[:], AF.Copy, scale=1.0 / GSIZE)
    var = smallp.tile([8, 4], F32, tag="var", name="var")
    nc.vector.tensor_tensor(var[:], mu[:, 0:4], mu[:, 0:4], ALU.mult)
    nc.vector.tensor_tensor(var[:], mu[:, 4:8], var[:], ALU.subtract)
    sd = smallp.tile([8, 4], F32, tag="sd", name="sd")
    nc.vector.tensor_scalar_add(sd[:], var[:], EPS)
    sdq = smallp.tile([8, 4], F32, tag="sdq", name="sdq")
    nc.scalar.activation(sdq[:], sd[:], AF.Sqrt)
    rs = smallp.tile([8, 4], F32, tag="rs", name="rs")
    nc.vector.reciprocal(rs[:], sdq[:])
    bc = pp.tile([P, 8], F32, tag="mm", name="bc")  # cols 0-3 mean, 4-7 rstd (bcast)
    nc.tensor.matmul(bc[:, 0:4], gbwd[:], mu[:, 0:4], start=True, stop=True)
    nc.tensor.matmul(bc[:, 4:8], gbwd[:], rs[:], start=True, stop=True)
    ga = smallp.tile([P, 4], F32, tag="ga", name="ga")  # per-channel scale
    gb = smallp.tile([P, 4], F32, tag="gb", name="gb")  # per-channel shift
    nc.vector.tensor_tensor(ga[:], consts[:, 4:8], bc[:, 4:8], ALU.mult)
    nc.vector.tensor_tensor(gb[:], bc[:, 0:4], ga[:], ALU.mult)
    nc.vector.tensor_tensor(gb[:], consts[:, 8:12], gb[:], ALU.subtract)

    # ---- GN apply -> h (bf16); hq first (gates Q), split across engines ----
    hsb = [hpool.tile([P, N], BF16, tag=f"h{m}", name=f"h{m}") for m in range(NCHUNK)]
    hq = [hpool.tile([P, NQ], BF16, tag=f"hq{m}", name=f"hq{m}") for m in range(NCHUNK)]
    for m in range(NCHUNK):
        nc.vector.tensor_scalar(hq[m][:], xq[m], ga[:, m:m + 1],
                                gb[:, m:m + 1], ALU.mult, ALU.add)
    for m in (0, 1):
        nc.scalar.activation(hsb[m][:], xsb[m], AF.Identity,
                             scale=ga[:, m:m + 1], bias=gb[:, m:m + 1])
    for m in (2, 3):
        nc.vector.tensor_scalar(hsb[m][:], xsb[m], ga[:, m:m + 1],
                                gb[:, m:m + 1], ALU.mult, ALU.add)

    # ---- Q projection into zero-padded head-pair layout ----
    # qpad[mo]: [128ch, 512] bf16; cols 0:256 head 2mo rows 0:64, cols
    # 256:512 head 2mo+1 rows 64:128, off-quadrants zero.
    qpad = [kvpool.tile([P, 2 * NQ], BF16, tag=f"qp{mo}", name=f"qp{mo}")
            for mo in range(NCHUNK)]
    for mo in range(NCHUNK):
        osl = slice(mo * 128, (mo + 1) * 128)
        pq = pp.tile([P, 1024], F32, tag="mm", name="pq")
        for ki in range(NCHUNK):
            nc.tensor.matmul(pq[:, 0:NQ], wq[ki][:, osl], hq[ki][:],
                             start=(ki == 0), stop=(ki == NCHUNK - 1))
        nc.gpsimd.memset(qpad[mo][0:64, NQ:2 * NQ], 0.0)
        nc.gpsimd.memset(qpad[mo][64:128, 0:NQ], 0.0)
        nc.vector.tensor_scalar_add(qpad[mo][0:64, 0:NQ], pq[0:64, 0:NQ],
                                    consts[0:64, mo:mo + 1])
        nc.vector.tensor_scalar_add(qpad[mo][64:128, NQ:2 * NQ],
                                    pq[64:128, 0:NQ],
                                    consts[64:128, mo:mo + 1])

    # ---- K projection (no bias: softmax shift-invariant), node-blocked so
    # the first score chunks (and hence the long ACT exp stream) start early.
    ksb = [kvpool.tile([P, N], BF16, tag=f"k{mo}", name=f"k{mo}") for mo in range(NCHUNK)]

    def k_block(nsl, on_act=False):
        width = nsl.stop - nsl.start
        for mo in range(NCHUNK):
            osl = slice(mo * 128, (mo + 1) * 128)
            pk = pp.tile([P, 1024], F32, tag="mm", name="pk")
            for ki in range(NCHUNK):
                nc.tensor.matmul(pk[:, 0:width], wk[ki][:, osl], hsb[ki][:, nsl],
                                 start=(ki == 0), stop=(ki == NCHUNK - 1))
            if on_act:
                nc.scalar.activation(ksb[mo][:, nsl], pk[:, 0:width], AF.Copy)
            else:
                nc.vector.tensor_copy(ksb[mo][:, nsl], pk[:, 0:width])

    # ---- per node-chunk: scores -> exp -> mask, V^T, then AV accumulate.
    # AV is jc-outer so each chunk's AV follows right behind its mask and
    # the PE never sits waiting for the full exp stream.  Two 256-col head
    # regions of po share each 2KB PSUM bank; start=True zeroes the whole
    # bank, so only the even head starts and only the odd head stops.
    asb = [apool.tile([P, H * NQ], BF16, tag=f"a{jc}", name=f"a{jc}")
           for jc in range(NJC)]
    vT = [kvpool.tile([P, H, DH + 1], BF16, tag=f"vT{jc}", name=f"vT{jc}")
          for jc in range(NJC)]
    po = ppo.tile([DH + 1, H * NQ], F32, tag="po", name="po")
    zinv = smallp.tile([1, H * NQ], F32, tag="zinv", name="zinv")
    zi16 = smallp.tile([1, H * NQ], BF16, tag="zi16", name="zi16")

    def chunk_block(jc):
        jsl = slice(jc * 128, (jc + 1) * 128)
        # V first: its PSUM buffer is freed by the fast DVE copy, so the
        # pool rotation never makes the PE wait on the slow exp reader of
        # the same chunk.
        pv = pp.tile([P, 1024], F32, tag="mm", name="pv")
        for ki in range(NCHUNK):
            nc.tensor.matmul(pv[:, 0:512], hsb[ki][:, jsl], wv[ki][:],
                             start=(ki == 0), stop=(ki == NCHUNK - 1))
        veng = nc.scalar if jc >= NJC - 2 else nc.vector
        if veng is nc.scalar:
            nc.scalar.activation(vT[jc][:, :, 0:DH],
                                 pv[:, 0:512].rearrange("p (h d) -> p h d", h=H),
                                 AF.Copy)
        else:
            nc.vector.tensor_copy(vT[jc][:, :, 0:DH],
                                  pv[:, 0:512].rearrange("p (h d) -> p h d", h=H))
        nc.gpsimd.memset(vT[jc][:, :, DH:DH + 1], 1.0)
        for pr in range(2):  # head quads (0-3 | 4-7) = mo pairs (0,1)|(2,3)
            ps = pp.tile([P, 1024], F32, tag="mm", name="ps")
            for mh in range(2):
                mo = pr * 2 + mh
                nc.tensor.matmul(ps[:, mh * 512:(mh + 1) * 512],
                                 ksb[mo][:, jsl], qpad[mo][:],
                                 start=True, stop=True)
            half = asb[jc][:, pr * 1024:(pr + 1) * 1024]
            nc.scalar.activation(half, ps[:], AF.Exp)
            nc.vector.tensor_tensor(
                half.rearrange("p (o q) -> p o q", o=4),
                half.rearrange("p (o q) -> p o q", o=4),
                msb[jc].rearrange("p (o q) -> p o q", o=1).broadcast_to(
                    [P, 4, NQ]),
                ALU.mult)
            # AV for this head quad right behind its mask
            for h in (4 * pr, 4 * pr + 1, 4 * pr + 2, 4 * pr + 3):
                nc.tensor.matmul(po[:, h * NQ:(h + 1) * NQ],
                                 vT[jc][:, h, :],
                                 asb[jc][:, h * NQ:(h + 1) * NQ],
                                 start=(jc == 0 and h % 2 == 0),
                                 stop=(jc == NJC - 1 and h % 2 == 1))
                if jc == NJC - 1 and h % 2 == 1:
                    # bank closed: launch its normalizer chain immediately;
                    # bf16 casts alternate ACT/Pool so they don't serialize
                    hp = h // 2
                    bsl = slice(hp * 2 * NQ, (hp + 1) * 2 * NQ)
                    nc.vector.reciprocal(zinv[:, bsl], po[DH:DH + 1, bsl])
                    if hp % 2 == 0:
                        nc.scalar.activation(zi16[:, bsl], zinv[:, bsl],
                                             AF.Copy)
                    else:
                        nc.gpsimd.tensor_copy(zi16[:, bsl], zinv[:, bsl])


    # ---- broadcast 1/Z machinery (osb channel layout: channel = h*64+d).
    # _emit_pz runs per head-pair, partly from inside the last chunk's AV
    # stream; the divides alternate DVE/Pool to halve the serial tail.
    _zstate = {}

    def _emit_pz(hp):
        if "zbc" not in _zstate:
            _zstate["zbc"] = smallp.tile([DH, H * NQ], BF16, tag="zbc",
                                         name="zbc")
            _zstate["osb"] = [opool.tile([P, NQ], BF16, tag=f"o{mo}",
                                         name=f"o{mo}")
                              for mo in range(NCHUNK)]
        zbc, osb = _zstate["zbc"], _zstate["osb"]
        bsl = slice(hp * 2 * NQ, (hp + 1) * 2 * NQ)
        pz = pp.tile([P, 1024], F32, tag="mm", name="pz")
        nc.tensor.matmul(pz[0:DH, 0:512], ones1[:], zi16[:, bsl],
                         start=True, stop=True)
        nc.scalar.activation(zbc[:, bsl], pz[0:DH, 0:512], AF.Copy)
        for h in (2 * hp, 2 * hp + 1):
            poff = (h % 2) * 64
            hsl = slice(h * NQ, (h + 1) * NQ)
            nc.vector.tensor_tensor(osb[h // 2][poff:poff + 64, :],
                                    po[0:DH, hsl], zbc[0:DH, hsl], ALU.mult)

    k_block(slice(0, 256), on_act=True)
    chunk_block(0)
    chunk_block(1)
    k_block(slice(256, 768))
    chunk_block(2)
    chunk_block(3)
    k_block(slice(768, 1024))
    for jc in range(4, NJC):
        chunk_block(jc)

    for hp in range(H // 2):
        _emit_pz(hp)
    osb = _zstate["osb"]

    # ---- output projection + bias + residual ----
    yall = opool.tile([P, NCHUNK, NQ], F32, tag="y", name="yall")
    for mo in range(NCHUNK):
        osl = slice(mo * 128, (mo + 1) * 128)
        py = pp.tile([P, 1024], F32, tag="mm", name="py")
        for ki in range(NCHUNK):
            nc.tensor.matmul(py[:, 0:NQ], wo[ki][:, osl], osb[ki][:],
                             start=(ki == 0), stop=(ki == NCHUNK - 1))
        nc.vector.scalar_tensor_tensor(yall[:, mo, :], py[:, 0:NQ],
                                       consts[:, 12 + mo:13 + mo],
                                       xq[mo], ALU.add, ALU.add)
    ydram = t["y"].rearrange("(m p) q -> p m q", p=P)
    nc.sync.dma_start(ydram[:, 0:2, :], yall[:, 0:2, :])
    nc.sync.dma_start(ydram[:, 2:4, :], yall[:, 2:4, :])


def _build():
    nc = bacc.Bacc("TRN2", target_bir_lowering=False, debug=False, num_devices=8)
    t = {}
    t["x16"] = nc.dram_tensor("x16", [C, N], BF16, kind="ExternalInput").ap()
    t["xq"] = nc.dram_tensor("xq", [C, NQ], F32, kind="ExternalInput").ap()
    t["mmask"] = nc.dram_tensor("mmask", [NJC, 128, NQ], BF16,
                                kind="ExternalInput").ap()
    for w in ("wqT", "wkT", "wvT", "woT"):
        t[w] = nc.dram_tensor(w, [C, C], BF16, kind="ExternalInput").ap()
    t["consts"] = nc.dram_tensor("consts", [128, 16], F32, kind="ExternalInput").ap()
    t["gfwd"] = nc.dram_tensor("gfwd", [128, 8], F32, kind="ExternalInput").ap()
    t["gbwd"] = nc.dram_tensor("gbwd", [8, 128], F32, kind="ExternalInput").ap()
    t["y"] = nc.dram_tensor("y", [C, NQ], F32, kind="ExternalOutput").ap()
    with tile.TileContext(nc) as tc, ExitStack() as ctx:
        t["ctx"] = ctx
        _emit(tc, nc, t)
    nc.compile()
    return nc


def _prep_inputs(inputs):
    x = np.asarray(inputs["x"], dtype=np.float32)
    idx = np.asarray(inputs["attend_idx"]).astype(np.int64)
    vm = np.asarray(inputs["valid_mask"]).astype(np.float32)
    wq = np.asarray(inputs["wq"], dtype=np.float32)
    wk = np.asarray(inputs["wk"], dtype=np.float32)
    wv = np.asarray(inputs["wv"], dtype=np.float32)
    wo = np.asarray(inputs["wo"], dtype=np.float32)
    bq = np.asarray(inputs["bq"], dtype=np.float32)
    bv = np.asarray(inputs["bv"], dtype=np.float32)
    bo = np.asarray(inputs["bo"], dtype=np.float32)
    gamma = np.asarray(inputs["gn_gamma"], dtype=np.float32)
    beta = np.asarray(inputs["gn_beta"], dtype=np.float32)

    cols = np.arange(C)
    perm = (cols % DH) * H + cols // DH   # wo_perm[:, h*64+d] = wo[:, d*8+h]
    wo_perm = wo[:, perm]
    bo_eff = bo + wo @ bv  # V-bias folds into output bias (softmax weights sum to 1)

    def colmajor(v):
        return np.ascontiguousarray(v.reshape(NCHUNK, 128).T)

    consts = np.concatenate(
        [colmajor(v) for v in (bq, gamma, beta, bo_eff)], axis=1)
    gfwd = np.zeros((128, 8), np.float32)
    gfwd[np.arange(128), np.arange(128) // 16] = 1.0
    gbwd = np.ascontiguousarray(gfwd.T)

    shared = {
        "wqT": np.ascontiguousarray(wq.T).astype(NPBF),
        "wkT": np.ascontiguousarray(wk.T).astype(NPBF),
        "wvT": np.ascontiguousarray(wv.T).astype(NPBF),
        "woT": np.ascontiguousarray(wo_perm.T).astype(NPBF),
        "consts": np.ascontiguousarray(consts),
        "gfwd": gfwd,
        "gbwd": gbwd,
    }
    x16 = [np.ascontiguousarray(x[b]).astype(NPBF) for b in range(B)]
    masks = []
    for qb in range(4):
        qs = slice(qb * NQ, (qb + 1) * NQ)
        Mr = np.zeros((N, NQ), np.float32)
        np.add.at(Mr, (idx[qs].ravel(), np.repeat(np.arange(NQ), K)),
                  vm[qs].ravel())
        masks.append(np.ascontiguousarray(Mr.reshape(NJC, 128, NQ)).astype(NPBF))
    in_maps = []
    for r in range(8):
        b, qb = r // 4, r % 4
        qs = slice(qb * NQ, (qb + 1) * NQ)
        m = dict(shared)
        m["x16"] = x16[b]
        m["xq"] = np.ascontiguousarray(x[b, :, qs])
        m["mmask"] = masks[qb]
        in_maps.append(m)
    return in_maps


def _get_runner(n_cores=8):
    """Build (once) a cached jitted SPMD executor mirroring
    bass2jax.run_bass_via_pjrt, so repeated calls don't re-trace."""
    if "runner" in _CACHE:
        return _CACHE["runner"]
    if "nc" not in _CACHE:
        _CACHE["nc"] = _build()
    nc = _CACHE["nc"]
    import jax
    from jax.sharding import Mesh, PartitionSpec
    from jax.experimental.shard_map import shard_map
    from concourse import bass2jax
    import concourse.mybir as _mybir

    bass2jax.install_neuronx_cc_hook()
    part_name = nc.partition_id_tensor.name if nc.partition_id_tensor else None
    in_names, out_names, out_avals, zero_outs = [], [], [], []
    for alloc in nc.m.functions[0].allocations:
        if not isinstance(alloc, _mybir.MemoryLocationSet):
            continue
        name = alloc.memorylocations[0].name
        if alloc.kind == "ExternalInput":
            if name != part_name:
                in_names.append(name)
        elif alloc.kind == "ExternalOutput":
            shape = tuple(alloc.tensor_shape)
            dtype = _mybir.dt.np(alloc.dtype)
            out_names.append(name)
            out_avals.append(jax.core.ShapedArray(shape, dtype))
            zero_outs.append(np.zeros(shape, dtype))
    n_params = len(in_names)
    n_outs = len(out_avals)
    all_names = in_names + out_names
    if part_name is not None:
        all_names = all_names + [part_name]
    donate = tuple(range(n_params, n_params + n_outs))

    def _body(*args):
        operands = list(args)
        if part_name is not None:
            operands.append(bass2jax.partition_id_tensor())
        outs = bass2jax._bass_exec_p.bind(
            *operands,
            out_avals=tuple(out_avals),
            in_names=tuple(all_names),
            out_names=tuple(out_names),
            lowering_input_output_aliases=(),
            sim_require_finite=True,
            sim_require_nnan=True,
            nc=nc,
        )
        return tuple(outs)

    devices = jax.devices()[:n_cores]
    mesh = Mesh(np.asarray(devices), ("core",))
    fn = jax.jit(
        shard_map(_body, mesh=mesh,
                  in_specs=(PartitionSpec("core"),) * (n_params + n_outs),
                  out_specs=(PartitionSpec("core"),) * n_outs,
                  check_rep=False),
        donate_argnums=donate, keep_unused=True)

    def run(in_maps, device_inputs=None):
        if device_inputs is None:
            device_inputs = put_inputs(in_maps)
        zo = [np.concatenate([np.zeros_like(z)] * n_cores, axis=0)
              for z in zero_outs]
        outs = fn(*device_inputs, *zo)
        outs = [np.asarray(o) for o in outs]
        split = [np.split(o, n_cores, axis=0) for o in outs]
        return [{name: split[i][c] for i, name in enumerate(out_names)}
                for c in range(n_cores)]

    def put_inputs(in_maps):
        cat = [np.concatenate([np.asarray(in_maps[c][nm])
                               for c in range(n_cores)], axis=0)
               for nm in in_names]
        return [jax.device_put(a) for a in cat]

    _CACHE["runner"] = (run, put_inputs, fn, n_params, n_outs)
    return _CACHE["runner"]


def _sim_fallback(nc, in_maps):
    """Correctness fallback if the PJRT/hardware path errors: run each
    core's shard through CoreSim."""
    from concourse.bass_interp import CoreSim
    results = []
    for m in in_maps:
        sim = CoreSim(nc, require_finite=False)
        for k, v in m.items():
            sim.tensor(k)[:] = v
        sim.simulate(check_with_hw=False)
        results.append({"y": np.array(sim.tensor("y"))})
    return results


def _assemble(results):
    out = np.empty((B, C, N), np.float32)
    for r in range(8):
        b, qb = r // 4, r % 4
        out[b, :, qb * NQ:(qb + 1) * NQ] = np.asarray(results[r]["y"])
    return out


def kernel(**inputs):
    in_maps = _prep_inputs(inputs)
    try:
        run, put_inputs, _, _, _ = _get_runner()
        results = run(in_maps)
    except Exception as e:
        sys.stderr.write(f"kernel: hardware path failed ({e!r}); "
                         "falling back to CoreSim\n")
        if "nc" not in _CACHE:
            _CACHE["nc"] = _build()
        results = _sim_fallback(_CACHE["nc"], in_maps)
    return _assemble(results)
